# revision 21
# baseline (speedup 1.0000x reference)
"""GAT self-attention Trainium2 kernel (v4).

Full inputs -> shard graphs over 8 NeuronCores -> full output.

Math (per graph n, reference reformulated):
  g_i = sigmoid(relu(q @ W1_i) @ W2_i)            [2d]
  u_i^L = W_i @ (g_i[:d] * a_i[:d])               [k]   (left projector)
  u_i^R = W_i @ (g_i[d:] * a_i[d:])               [k]   (right projector)
  l_i = X @ u_i^L ; r_i = X @ u_i^R               [E]
  S[i,j] = lrelu(l_t[i] + r_t[j]), t = adj[i,j]
  E' = exp(S) * (adj > 0); rs = rowsum(E')
  h = X @ W_2 ; out = (E'/rs)^T @ h

v4 notes (PE sequencer + PE clock-ramp were the bottlenecks):
  - prep gate chain in wide-free orientation: graphs on PSUM partitions,
    weights as the moving operand (free=512 halves; matmul PSUM out is
    one bank max).  ~200 PE instructions instead of ~400.
  - prep software-pipelined across types so PE never idles long enough
    to drop out of its pstate ramp; intermediates live in 1-bank PSUM
    half-tiles from a prep-scoped pool that frees before the main-loop
    score pools allocate.
  - a_type folded into the gv transpose copy-out; U^L/U^R share matmuls
    (rhs = vT[:, dc::4, :]) writing U6 in its final orientation.
  - LR + stack scatter run one graph ahead of the score chunks so the
    rank-2 score matmuls never wait on the DMA round trip.
  - DMA routing: Xt/adj/out on SP HWDGE, weights + scatters on Act
    HWDGE; nothing on Pool SWDGE.  Masks (bf16 mz, u8 m2/m3) on gpsimd,
    select + softmax chain on DVE, prelu/exp + h/out copies on Act.
"""
import numpy as np
from contextlib import ExitStack

import concourse.bass as bass
import concourse.tile as tile
from concourse import mybir, bacc
from concourse.masks import make_identity

F32 = mybir.dt.float32
F32R = mybir.dt.float32r
BF16 = mybir.dt.bfloat16
U8 = mybir.dt.uint8
AF = mybir.ActivationFunctionType
OP = mybir.AluOpType

N_CORES = 8
N, E, K, D = 64, 512, 512, 512   # graphs, entities, in_dim, out_dim
NG = N // N_CORES                # graphs per core
NT = 3                           # edge types
P = 128
EC = E // P                      # 4 partition chunks of E
KC = K // P
DC2 = (2 * D) // P               # 8 chunks of the 2d gate dim


def build(nc, reps=1):
    x = nc.dram_tensor("x", [NG, E, K], BF16, kind="ExternalInput").ap()
    adjf = nc.dram_tensor("adjf", [NG, E, E], BF16, kind="ExternalInput").ap()
    qv = nc.dram_tensor("qv", [NG, K], F32, kind="ExternalInput").ap()
    Wt = nc.dram_tensor("Wt", [NT, K, D], BF16, kind="ExternalInput").ap()
    at = nc.dram_tensor("at", [NT, 2 * D], F32, kind="ExternalInput").ap()
    W1 = nc.dram_tensor("W1", [NT, K, 2 * D], BF16, kind="ExternalInput").ap()
    W2q = nc.dram_tensor("W2q", [NT, 2 * D, 2 * D], BF16, kind="ExternalInput").ap()
    out = nc.dram_tensor("out", [NG, E, D], BF16, kind="ExternalOutput").ap()
    nc._gat_io = (x, adjf, qv, Wt, at, W1, W2q, out)
    _build_once(nc, reps)


def _build_once(nc, reps=1):
    x, adjf, qv, Wt, at, W1, W2q, out = nc._gat_io
    with tile.TileContext(nc) as tc, ExitStack() as ctx:
        # ---------------- persistent tiles ----------------
        pers = ctx.enter_context(tc.tile_pool(name="pers", bufs=1))
        identb = pers.tile([P, P], BF16)
        make_identity(nc, identb[:])
        identf = pers.tile([P, P], F32)
        make_identity(nc, identf[:])
        # U6[k%128, kc, side, t, n]; LR lhsT = U6[:, kc, :, :, n] (free 6)
        U6 = pers.tile([P, KC, 2, NT, NG], BF16)
        Wt2_sb = pers.tile([P, KC, D], BF16)
        nc.scalar.dma_start(Wt2_sb[:], Wt[2].rearrange("(c p) d -> p c d", p=P))
        aT = pers.tile([P, DC2, NT], F32)
        # Score-stack tiles: operand pair for type t at base partition 32*t
        # (compute engines may only touch partition ranges based at 0/32/64/96).
        # lhsT_t = stkL[32t:32t+2] = [1s; l_t]; rhs_t = stkR[32t:32t+2] =
        # [r_t; 1s].  A/B buffering across graphs.  Full-tile zero memset so
        # the fp32r 4-partition read granule never touches uninit memory.
        ones3 = pers.tile([NT, E], F32)
        nc.vector.memset(ones3[:], 1.0)
        stkL = [pers.tile([66, E], F32, name=f"stkL{i}") for i in range(2)]
        stkR = [pers.tile([66, E], F32, name=f"stkR{i}") for i in range(2)]
        for sb in range(2):
            nc.vector.memset(stkL[sb][:, :], 0.0)
            nc.vector.memset(stkR[sb][:, :], 0.0)
            for t in range(NT):
                nc.vector.memset(stkL[sb][32 * t:32 * t + 1, :], 1.0)
        for sb in range(2):
            nc.sync.dma_start(stkR[sb][1:66:32, :], ones3[:])

        # h-psum pool exists for the whole kernel (loads run during prep)
        ps_h = ctx.enter_context(tc.tile_pool(name="ps_h", bufs=1, space="PSUM"))

        # ---------------- sbuf pools ----------------
        deep = ctx.enter_context(tc.tile_pool(name="deep", bufs=2))
        p_adj = ctx.enter_context(tc.tile_pool(name="p_adj", bufs=4))
        p_xt = ctx.enter_context(tc.tile_pool(name="p_xt", bufs=4))
        p_hs = ctx.enter_context(tc.tile_pool(name="p_hs", bufs=4))
        p_msk = ctx.enter_context(tc.tile_pool(name="p_msk", bufs=3))
        sbuf = ctx.enter_context(tc.tile_pool(name="sbuf", bufs=2))
        small = ctx.enter_context(tc.tile_pool(name="small", bufs=3))

        def load_graph(n):
            adj_sb = p_adj.tile([P, EC, E], BF16, tag="adj")
            Xt_sb = p_xt.tile([P, KC, E], BF16, tag="Xt")
            nc.sync.dma_start_transpose(Xt_sb[:], x[n])
            nc.sync.dma_start(adj_sb[:], adjf[n].rearrange("(c p) j -> p c j", p=P))
            # masks on gpsimd (Pool has no other work; can't touch PSUM)
            mz = p_msk.tile([P, EC, E], BF16, tag="mz")
            nc.gpsimd.tensor_scalar(mz[:], adj_sb[:], 0.5, None, OP.is_gt)
            m2 = p_msk.tile([P, EC, E], U8, tag="m2")
            nc.gpsimd.tensor_scalar(m2[:], adj_sb[:], 2.0, None, OP.is_equal)
            m3 = p_msk.tile([P, EC, E], U8, tag="m3")
            nc.gpsimd.tensor_scalar(m3[:], adj_sb[:], 3.0, None, OP.is_equal)
            # h = X @ W_2 (unscaled); psum copy-out split DVE/Act
            h_sb = p_hs.tile([P, EC, D], BF16, tag="hs")
            for ic in range(EC):
                pH = ps_h.tile([P, D], F32, tag="ph")
                for kc in range(KC):
                    nc.tensor.matmul(pH[:], Xt_sb[:, kc, ic * P:(ic + 1) * P],
                                     Wt2_sb[:, kc, :],
                                     start=(kc == 0), stop=(kc == KC - 1))
                if ic < 2:
                    nc.vector.tensor_copy(h_sb[:, ic, :], pH[:])
                else:
                    nc.scalar.copy(h_sb[:, ic, :], pH[:])
            return dict(Xt_sb=Xt_sb, h_sb=h_sb, mz=mz, m2=m2, m3=m3)

        # ---------------- prep: gates + projector vectors ----------------
        def run_prep():
          with tc.tile_pool(name="prep", bufs=2) as prep, \
               tc.tile_pool(name="pp", bufs=4, space="PSUM") as pp, \
               tc.tile_pool(name="pt", bufs=2, space="PSUM") as pt:
            # DMA chain order matters: every HWDGE DMA completion is
            # serialized, so emit in need-order — gate weights first.
            qv_nat = prep.tile([NG, K], F32, tag="qn", bufs=1)
            nc.scalar.dma_start(qv_nat[:], qv)
            at_nat = prep.tile([NT, 2 * D], F32, tag="an", bufs=1)
            nc.scalar.dma_start(at_nat[:], at)
            W1s, W2s, WTs = [], [], []
            for i in range(NT):
                W1_sb = prep.tile([P, KC, 2 * D], BF16, name=f"W1_{i}", tag="w1")
                nc.scalar.dma_start(W1_sb[:], W1[i].rearrange("(c p) f -> p c f", p=P))
                W1s.append(W1_sb)
                W2_sb = prep.tile([P, DC2, 2 * D], BF16, name=f"W2_{i}", tag="w2")
                nc.scalar.dma_start(
                    W2_sb[:], W2q[i].rearrange("(c p) f -> p c f", p=P))
                W2s.append(W2_sb)
                WTi = prep.tile([P, EC, K], BF16, name=f"WT_{i}", tag="wt", bufs=3)
                nc.sync.dma_start_transpose(WTi[:], Wt[i])
                WTs.append(WTi)
            # qT[k%128, kc, n] via PE transposes batched in one PSUM tile
            qT = prep.tile([P, KC, NG], BF16, tag="qT", bufs=1)
            qps = pt.tile([P, E], F32, tag="pt")
            for kc in range(KC):
                nc.tensor.transpose(
                    qps[:, kc * NG:(kc + 1) * NG],
                    qv_nat[:, kc * P:(kc + 1) * P], identf[:NG, :NG])
            nc.vector.tensor_copy(
                qT[:], qps[:, 0:KC * NG].rearrange("p (k n) -> p k n", k=KC))
            # aT[d2%128, oc, t] via PE transposes batched in one PSUM tile
            aps = pt.tile([P, E], F32, tag="pt")
            for oc in range(DC2):
                nc.tensor.transpose(
                    aps[:, oc * NT:(oc + 1) * NT],
                    at_nat[:, oc * P:(oc + 1) * P], identf[:NT, :NT])
            nc.vector.tensor_copy(
                aT[:], aps[:, 0:DC2 * NT].rearrange("p (c t) -> p c t", c=DC2))

            # -- pipeline stages (each returns tiles for the next stage) --
            def rr_mm(t):
                hs = []
                for hf in range(2):
                    ph = pp.tile([NG, D], F32, tag="pp", name=f"rr{t}h{hf}")
                    for kc in range(KC):
                        nc.tensor.matmul(ph[:], qT[:, kc, :],
                                         W1s[t][:, kc, hf * D:(hf + 1) * D],
                                         start=(kc == 0), stop=(kc == KC - 1))
                    hs.append(ph)
                return hs

            def rr_act(t, hs):
                rrb = prep.tile([NG, 2 * D], BF16, tag="rrb")
                nc.scalar.activation(rrb[:, 0:D], hs[0][:], AF.Relu)
                nc.scalar.activation(rrb[:, D:2 * D], hs[1][:], AF.Relu)
                return rrb

            def rr_T(t, rrb):
                ptr = pt.tile([P, DC2 * NG], BF16, tag="pt")
                for dc in range(DC2):
                    nc.tensor.transpose(
                        ptr[:, dc * NG:(dc + 1) * NG],
                        rrb[:, dc * P:(dc + 1) * P], identb[:NG, :NG])
                rrT = prep.tile([P, DC2, NG], BF16, tag="rrT")
                nc.vector.tensor_copy(
                    rrT[:], ptr[:, 0:DC2 * NG].rearrange("p (c n) -> p c n", c=DC2))
                return rrT

            def gv_mm(t, rrT):
                hs = []
                for hf in range(2):
                    ph = pp.tile([NG, D], F32, tag="pp", name=f"gv{t}h{hf}")
                    for dc in range(DC2):
                        nc.tensor.matmul(ph[:], rrT[:, dc, :],
                                         W2s[t][:, dc, hf * D:(hf + 1) * D],
                                         start=(dc == 0), stop=(dc == DC2 - 1))
                    hs.append(ph)
                return hs

            def gv_act(t, hs):
                gvb = prep.tile([NG, 2 * D], BF16, tag="gvb")
                nc.scalar.activation(gvb[:, 0:D], hs[0][:], AF.Sigmoid)
                nc.scalar.activation(gvb[:, D:2 * D], hs[1][:], AF.Sigmoid)
                return gvb

            def gv_T(t, gvb):
                ptg = pt.tile([P, DC2 * NG], BF16, tag="pt")
                for dc in range(DC2):
                    nc.tensor.transpose(
                        ptg[:, dc * NG:(dc + 1) * NG],
                        gvb[:, dc * P:(dc + 1) * P], identb[:NG, :NG])
                vT = prep.tile([P, DC2, NG], BF16, tag="vT")
                for dc in range(DC2):
                    nc.vector.tensor_scalar(
                        vT[:, dc, :], ptg[:, dc * NG:(dc + 1) * NG],
                        aT[:, dc, t:t + 1], None, OP.mult)
                return vT

            def u_mm(t, vT):
                pu = pt.tile([P, KC, 2, NG], F32, tag="pt")
                for kc in range(KC):
                    for dc in range(EC):
                        nc.tensor.matmul(
                            pu[:, kc, :, :],
                            WTs[t][:, dc, kc * P:(kc + 1) * P],
                            vT[:, dc::EC, :],
                            start=(dc == 0), stop=(dc == EC - 1))
                nc.vector.tensor_copy(U6[:, :, :, t, :], pu[:])

            # software pipeline across the 3 types
            A = rr_mm(0)
            rrb0 = rr_act(0, A)
            B = rr_mm(1)
            rrT0 = rr_T(0, rrb0)
            G0 = gv_mm(0, rrT0)
            rrb1 = rr_act(1, B)
            gvb0 = gv_act(0, G0)
            C = rr_mm(2)
            rrT1 = rr_T(1, rrb1)
            vT0 = gv_T(0, gvb0)
            u_mm(0, vT0)
            G1 = gv_mm(1, rrT1)
            rrb2 = rr_act(2, C)
            gvb1 = gv_act(1, G1)
            rrT2 = rr_T(2, rrb2)
            vT1 = gv_T(1, gvb1)
            u_mm(1, vT1)
            G2 = gv_mm(2, rrT2)
            gvb2 = gv_act(2, G2)
            vT2 = gv_T(2, gvb2)
            u_mm(2, vT2)

        # ---------------- main loop ----------------
        # prep first: its DMAs lead the serialized DMA chain and its PE ops
        # lead the PE queue (h-matmuls for graphs 0/1 can run late; they are
        # only needed once the first score chunks finish).
        st = {}
        LOOKAHEAD = 2
        run_prep()
        for n in range(LOOKAHEAD):
            st[n] = load_graph(n)

        ps_s1 = ctx.enter_context(tc.tile_pool(name="ps_s1", bufs=2, space="PSUM"))
        ps_s23 = ctx.enter_context(tc.tile_pool(name="ps_s23", bufs=1, space="PSUM"))
        ps_o = ctx.enter_context(tc.tile_pool(name="ps_o", bufs=2, space="PSUM"))
        ps_lr = ctx.enter_context(tc.tile_pool(name="ps_lr", bufs=1, space="PSUM"))

        def emit_lr(n):
            # LR rows -> score stacks, one graph ahead of its score chunks
            pLR = ps_lr.tile([2 * NT, E], F32, tag="lr")
            Xt_sb = st[n]["Xt_sb"]
            for kc in range(KC):
                nc.tensor.matmul(pLR[:], U6[:, kc, :, :, n], Xt_sb[:, kc, :],
                                 start=(kc == 0), stop=(kc == KC - 1))
            sL, sR = stkL[n % 2], stkR[n % 2]
            stg = small.tile([2 * NT, E], F32, tag="stg")
            nc.vector.tensor_copy(stg[:], pLR[:])
            # l_t -> row 32t+1 of stkL; r_t -> row 32t of stkR (DMA scatter)
            nc.scalar.dma_start(sL[1:66:32, :], stg[0:NT, :])
            nc.scalar.dma_start(sR[0:65:32, :], stg[NT:2 * NT, :])

        def compute_graph(n, stn):
            h_sb = stn["h_sb"]
            mz, m2, m3 = stn["mz"], stn["m2"], stn["m3"]
            sL, sR = stkL[n % 2], stkR[n % 2]

            rs = small.tile([P, EC], F32, tag="rs")
            rsr = small.tile([P, EC], F32, tag="rsr")
            E_sb = deep.tile([P, EC, E], BF16, tag="E")
            for ic in range(EC):
                pv1 = ps_s1.tile([P, E], F32, tag="s1")
                nc.tensor.matmul(
                    pv1[:], sL[0:2, ic * P:(ic + 1) * P].bitcast(F32R),
                    sR[0:2, :].bitcast(F32R), start=True, stop=True)
                pv23 = ps_s23.tile([P, 2, E], F32, tag="s23")
                nc.tensor.matmul(
                    pv23[:, 0, :], sL[32:34, ic * P:(ic + 1) * P].bitcast(F32R),
                    sR[32:34, :].bitcast(F32R), start=True, stop=True)
                nc.tensor.matmul(
                    pv23[:, 1, :], sL[64:66, ic * P:(ic + 1) * P].bitcast(F32R),
                    sR[64:66, :].bitcast(F32R), start=True, stop=True)
                nc.vector.copy_predicated(pv1[:], m2[:, ic, :], pv23[:, 0, :])
                nc.vector.copy_predicated(pv1[:], m3[:, ic, :], pv23[:, 1, :])
                lr_sb = small.tile([P, E], BF16, tag="lrl")
                nc.scalar.activation(lr_sb[:], pv1[:], AF.Prelu, alpha=0.2)
                e1_sb = small.tile([P, E], BF16, tag="e1")
                nc.scalar.activation(e1_sb[:], lr_sb[:], AF.Exp)
                # E' = e1 * (adj>0), rowsum into rs
                nc.vector.scalar_tensor_tensor(
                    E_sb[:, ic, :], e1_sb[:], 1.0, mz[:, ic, :],
                    OP.mult, OP.mult, accum_out=rs[:, ic:ic + 1])
                nc.vector.reciprocal(rsr[:, ic:ic + 1], rs[:, ic:ic + 1])
                # softmax normalization: scale E' rows in place (DVE 4x)
                nc.vector.tensor_scalar(E_sb[:, ic, :], E_sb[:, ic, :],
                                        rsr[:, ic:ic + 1], None, OP.mult)

            # ---- out = coef^T @ h ----
            out_sb = sbuf.tile([P, EC, D], BF16, tag="osb")
            for jc in range(EC):
                pO = ps_o.tile([P, D], F32, tag="po")
                for ic in range(EC):
                    nc.tensor.matmul(pO[:], E_sb[:, ic, jc * P:(jc + 1) * P],
                                     h_sb[:, ic, :],
                                     start=(ic == 0), stop=(ic == EC - 1))
                nc.scalar.copy(out_sb[:, jc, :], pO[:])
            nc.sync.dma_start(out[n].rearrange("(c p) d -> p c d", p=P), out_sb[:])

        emit_lr(0)
        for n in range(NG):
            if n + LOOKAHEAD < NG:
                st[n + LOOKAHEAD] = load_graph(n + LOOKAHEAD)
            if n + 1 < NG:
                emit_lr(n + 1)
            compute_graph(n, st.pop(n))
    return nc


_NC_CACHE = {}
TRACE = False
_LAST = {}


def _get_nc():
    if "nc" not in _NC_CACHE:
        nc = bacc.Bacc("TRN2", target_bir_lowering=False, debug=False)
        build(nc)
        nc.compile()
        _NC_CACHE["nc"] = nc
    return _NC_CACHE["nc"]


def kernel(input_state, adj, entity_mask, query_vec, W_type, a_type,
           qattn_W1, qattn_W2):
    from concourse import bass_utils
    import ml_dtypes
    bf16 = ml_dtypes.bfloat16
    nc = _get_nc()
    x = np.ascontiguousarray(input_state).astype(bf16)
    adjf = np.ascontiguousarray(adj).astype(bf16)
    qvf = np.ascontiguousarray(query_vec, dtype=np.float32)
    Wt = np.ascontiguousarray(W_type).astype(bf16)
    at = np.ascontiguousarray(a_type, dtype=np.float32)
    W1 = np.ascontiguousarray(qattn_W1).astype(bf16)
    W2q = np.ascontiguousarray(qattn_W2).astype(bf16)

    in_maps = []
    for c in range(N_CORES):
        sl = slice(c * NG, (c + 1) * NG)
        in_maps.append({
            "x": x[sl], "adjf": adjf[sl], "qv": qvf[sl],
            "Wt": Wt, "at": at, "W1": W1, "W2q": W2q,
        })
    res = bass_utils.run_bass_kernel_spmd(nc, in_maps, core_ids=list(range(N_CORES)),
                                          trace=TRACE, stitch_traces=TRACE)
    _LAST["exec_ns"] = res.exec_time_ns
    _LAST["mean_ns"] = res.mean_exec_time_ns
    _LAST["trace"] = res.instructions_and_trace
    out = np.concatenate([r["out"] for r in res.results], axis=0)
    return out.astype(np.float32)
